# revision 25
# baseline (speedup 1.0000x reference)
"""Softsign multi-head attention on 8 Trainium2 NeuronCores (Bass/Tile).

Host<->device IO is minimized on both axes (it dominates the graded time):
42.1 MB total moved (vs 198 MB naive) in just 2 transfers per core.

  core c = 2*b + hg   (batch b of 4, head-group hg of 2 -> 8 heads each)
  Every unique byte is uploaded to exactly ONE core as float16, packed
  into a single [1540, E] blob per core (wq/wk/wv quarters | wo quarter |
  x half | f32 bias bytes | consts), then replicated device-side over
  NeuronLink collectives:
  - x[b]: split by sequence half across the pair (2b, 2b+1), pairwise
    AllGather'd in two chunks               (x H2D: 16.8 MB total, was 64)
  - head-group W slices: split in 4 across {hg, 2+hg, 4+hg, 6+hg},
    quad-AllGather'd (+1 regroup DMA)       (W H2D:  8.4 MB total, was 64)
  - the two partial outputs per batch are summed on device via pairwise
    ReduceScatter (chunked per S/4 block)   (out D2H: 16.8 MB total, was 64)

No host-side transposes: x and W arrive in natural layout and are
transposed on-chip by XBAR transpose-DMAs (DRAM -> SBUF, fp16).

Softsign(s) = s/(1+|s|) is ONE single-input custom DVE op per tile
(8 ALU stages): t = s|-0.0 = -|s|; u = 1-t; nu = bitcast(~u);
y ~= 1/u via minimax on W=u*nu; out = s*y  (max rel err ~1.7e-3).
"""

import sys

sys.path.insert(0, "/opt/trn_rl_repo")

import numpy as np

import concourse.bass as bass
import concourse.dve_ops as dve_ops
import concourse.mybir as mybir
import concourse.tile as tile
from concourse.bass_utils import run_bass_kernel_spmd
from concourse.dve_ops import DveOp
from concourse.dve_spec import AluOp, Bin, One, Spec, Src0, C0, C1, C2, lower
from concourse.dve_uop import DveOpSpec

f32 = mybir.dt.float32
f16 = mybir.dt.float16
AF = mybir.ActivationFunctionType

# ---------------------------------------------------------------- softsign op
A_CONST = -0.4714038456062873
B_CONST = 0.055459279842660344


def _ref_softsign1(in0, in1, s0, s1, imm2):
    s = in0.astype(np.float32)
    t = (s.view(np.uint32) | np.float32(imm2).view(np.uint32)).view(np.float32)
    u = (np.float32(1.0) - t).astype(np.float32)
    nu = (~u.view(np.int32)).view(np.float32)
    W = (u * nu).astype(np.float32)
    y1 = (nu * (np.float32(s0) - W * np.float32(s1))).astype(np.float32)
    return (s * y1).astype(np.float32)


def _register_softsign() -> DveOp:
    for existing in dve_ops.OPS:
        if existing.name == "SOFTSIGN_AO_ANT":
            return existing
    t = Bin(AluOp.BITWISE_OR, Src0, C2)          # -|s|
    u = Bin(AluOp.SUBTRACT, One, t)              # 1+|s|
    nu = Bin(AluOp.BITWISE_NOT, u, u)
    W = u * nu
    body = Src0 * (nu * (C0 - W * C1))
    spec = Spec(body=body, reference=_ref_softsign1)
    shas = {}
    for ver in ("v3", "v4"):
        uops = lower(spec, ver=ver)
        tmp = DveOpSpec(name="SOFTSIGN_AO_ANT", opcode=31, uops=uops, rd1_en=False)
        shas[ver] = tmp.sha(ver)
    op = DveOp("SOFTSIGN_AO_ANT", spec, subdim=False, uops_sha=shas)
    dve_ops.OPS.append(op)
    dve_ops.CUSTOM_DVE_SPECS[op.name] = op.spec
    dve_ops._SUB_OPCODE_FOR_NAME[op.name] = (
        dve_ops._CUSTOM_DVE_ROW_BASE + len(dve_ops.OPS) - 1
    )
    return op


def _emit_softsign(nc, out, s):
    op = _register_softsign()
    return nc.vector._custom_dve(
        op, out=out, in0=s, s0=A_CONST, s1=B_CONST, imm2=-0.0
    )


# ------------------------------------------------------------- wait splitting
_ws_ctr = [0]


def _split_excess_waits(nc, limit=1):
    """This container's walrus accepts a single sync-wait command per
    instruction; push excess waits onto prefix NoOps on the same engine."""
    for f in nc.m.functions:
        for b in f.blocks:
            new_insts = []
            for inst in b.instructions:
                si = getattr(inst, "sync_info", None)
                ow = list(si.on_wait) if si and si.on_wait else []
                if len(ow) > limit:
                    excess, keep = ow[:-limit], ow[-limit:]
                    for i in range(0, len(excess), limit):
                        chunk = excess[i : i + limit]
                        _ws_ctr[0] += 1
                        nop = mybir.InstNoOp(
                            name=f"waitsplit-{_ws_ctr[0]}",
                            ins=[],
                            outs=[],
                            engine=inst.engine,
                            sync_info=mybir.SyncInfo(on_wait=chunk, on_update=[]),
                            text_hint="waitsplit",
                        )
                        nc.register_instruction(nop, overwrite=True)
                        new_insts.append(nop)
                    si.on_wait = keep
                new_insts.append(inst)
            b.instructions = new_insts


# --------------------------------------------------------------- kernel build
S, E, F, D = 2048, 1024, 512, 64
NE, NF, NS, NST, GRP = 8, 4, 4, 16, 2
SH = S // 2  # per-core sequence half
PAIRS = [[0, 1], [2, 3], [4, 5], [6, 7]]
QUADS = [[0, 2, 4, 6], [1, 3, 5, 7]]
BYP = mybir.AluOpType.bypass
ADD = mybir.AluOpType.add


def _build(reps=1):
    _register_softsign()
    nc = bass.Bass(num_devices=8)
    # ONE fp16 input blob per core: rows [0,384)=wqkv quarters,
    # [384,512)=wo quarter ([256,512] bytes reinterpreted), [512,1536)=x
    # half, [1536,1538)=bqk bytes ([128,8] f32), [1538,1540)=consts bytes
    # ([1,2048] f16: ones(128)|bv(512)|bo(1024)|pad)
    blob_d = nc.declare_dram_parameter("blob", [1540, E], f16, isOutput=False)
    out_d = nc.declare_dram_parameter("out", [SH, E], f16, isOutput=True)
    if reps > 1:
        # reps-dependent shape marker: busts the axon executable-loader
        # dedup (keyed on module name + I/O shapes) so a reps>1 build runs
        # its own NEFF in the repeated-body timing method. Production
        # (reps=1) omits it - one fewer host->device transfer.
        mark_d = nc.declare_dram_parameter(
            "marker", [reps, 1], f32, isOutput=False)

    with tile.TileContext(nc) as tc:
        with (
            tc.tile_pool(name="persist", bufs=1) as pp,
            tc.tile_pool(name="dram", bufs=1, space="DRAM") as dp,
        ):
            # ---- DRAM bounce buffers for collectives
            wqkvL = dp.tile([384, E], f16, tag="wqkvL")
            woL = dp.tile([128, E], f16, tag="woL")
            xL = dp.tile([SH, E], f16, tag="xL")
            wG = dp.tile([1536, E], f16, tag="wG")
            wR = dp.tile([1536, E], f16, tag="wR")
            woF = dp.tile([E, F], f16, tag="woF")
            # xG[h]: pair-AllGather of x rows [512h, 512h+512) of each half;
            # xG[h][0:512] = member-0 chunk (ss=2h), [512:1024] = member-1
            # chunk (ss=2h+1 ... i.e. global rows 1024+512h).
            xG = [dp.tile([1024, E], f16, tag=f"xG{h}", name=f"xG{h}")
                  for h in range(2)]
            ob = [dp.tile([512, E], f16, tag=f"ob{i}", name=f"ob{i}")
                  for i in range(NS)]
            obr = [dp.tile([256, E], f16, tag=f"obr{i}", name=f"obr{i}")
                   for i in range(NS)]

            # input -> bounce on the HWDGE queue (keeps the CC queue clear),
            # then device-side replication.
            nc.sync.dma_start(xL[0:512, :], blob_d[512:1024, :])
            nc.sync.dma_start(xL[512:1024, :], blob_d[1024:1536, :])
            nc.sync.dma_start(wqkvL[:], blob_d[0:384, :])
            nc.sync.dma_start(woL[:], blob_d[384:512, :])
            # weights-first: their H2D lands before x's (declaration order),
            # so these collectives hide under x's transfer tail instead of
            # queuing behind a blocked x collective.
            nc.gpsimd.collective_compute(
                "AllGather", BYP, QUADS, ins=[wqkvL.opt()], outs=[wG.opt()])
            nc.gpsimd.collective_compute(
                "AllGather", BYP, QUADS, ins=[woL.opt()], outs=[woF.opt()])
            nc.gpsimd.collective_compute(
                "AllGather", BYP, PAIRS,
                ins=[xL[0:512, :].opt()], outs=[xG[0].opt()])
            nc.gpsimd.collective_compute(
                "AllGather", BYP, PAIRS,
                ins=[xL[512:1024, :].opt()], outs=[xG[1].opt()])
            # regroup wG rows (member-major) -> matrix-major wR:
            # wR = [wqF(512); wkF(512); wvF(512)], one 12-descriptor DMA
            wG4 = wG[:].rearrange("(m q r) e -> q m (r e)", m=4, q=3, r=128)
            for q in range(3):
                nc.scalar.dma_start(
                    wR[512 * q:512 * (q + 1), :].rearrange(
                        "(m r) e -> m (r e)", m=4, r=128),
                    wG4[q],
                )

            # ---- persistent SBUF
            wTq = [pp.tile([128, F], f16, tag=f"wTq{e}", name=f"wTq{e}")
                   for e in range(NE)]
            wTk = [pp.tile([128, F], f16, tag=f"wTk{e}", name=f"wTk{e}")
                   for e in range(NE)]
            wTv = [pp.tile([128, F], f16, tag=f"wTv{e}", name=f"wTv{e}")
                   for e in range(NE)]
            woT = [pp.tile([128, E], f16, tag=f"woT{t}", name=f"woT{t}")
                   for t in range(NF)]
            q_sb = [pp.tile([128, S], f16, tag=f"q{t}", name=f"q{t}")
                    for t in range(NF)]
            k_sb = [pp.tile([128, S], f16, tag=f"k{t}", name=f"k{t}")
                    for t in range(NF)]
            v_sb = [pp.tile([128, F], f16, tag=f"v{t}", name=f"v{t}")
                    for t in range(NST)]
            ctx_sb = [pp.tile([128, S], f16, tag=f"c{t}", name=f"c{t}")
                      for t in range(NF)]
            bqk_sb = pp.tile([128, 2 * NF], f32, tag="bqk")
            cst_sb = pp.tile([1, 2048], f16, tag="consts")
            nc.sync.dma_start(
                bqk_sb[:], blob_d[1536:1538, :].bitcast(f32))
            nc.sync.dma_start(cst_sb[:], blob_d[1538:1540, :])
            bq_sb = bqk_sb[:, 0:NF]
            bk_sb = bqk_sb[:, NF:2 * NF]
            ones_sb = cst_sb[:, 0:128]
            bv_sb = cst_sb[:, 128:128 + F]
            bo_sb = cst_sb[:, 128 + F:128 + F + E]
            if reps > 1:
                mark_sb = pp.tile([reps, 1], f32, tag="marker")
                nc.sync.dma_start(mark_sb[:], mark_d[:])

            for _rep in range(reps):
                # ---------- Phase W: XBAR transpose-DMA of weights ----------
                for mi, wT in enumerate((wTq, wTk, wTv)):
                    for e in range(NE):
                        nc.sync.dma_start_transpose(
                            wT[e][:],
                            wR[512 * mi:512 * (mi + 1),
                               e * 128:(e + 1) * 128])
                for fch in range(NF):
                    nc.sync.dma_start_transpose(
                        woT[fch][:], woF[:, fch * 128:(fch + 1) * 128])

                # ---------- Phase 1: x transpose + q/k/v projections --------
                with (
                    tc.tile_pool(name=f"xt{_rep}", bufs=2) as xtp,
                    tc.tile_pool(name=f"psA{_rep}", bufs=2, space="PSUM") as psA,
                    tc.tile_pool(name=f"psB{_rep}", bufs=2, space="PSUM") as psB,
                ):
                    for ss in (0, 2, 1, 3):
                        sl = slice(ss * 512, (ss + 1) * 512)
                        xsrc = xG[ss % 2]
                        xr0 = 512 * (ss // 2)
                        xt = []
                        for e in range(NE):
                            t = xtp.tile([128, 512], f16, tag=f"xt{e}",
                                         name=f"xt{e}")
                            nc.sync.dma_start_transpose(
                                t[:], xsrc[xr0:xr0 + 512,
                                           e * 128:(e + 1) * 128])
                            xt.append(t)
                        for wT, dst, b_sb in ((wTq, q_sb, bq_sb),
                                              (wTk, k_sb, bk_sb)):
                            for ft in range(NF):
                                ps = psA.tile([128, 512], f32, tag="proj",
                                              name="psproj")
                                for e in range(NE):
                                    nc.tensor.matmul(
                                        ps[:],
                                        wT[e][:, ft * 128:(ft + 1) * 128],
                                        xt[e][:],
                                        start=(e == 0), stop=(e == NE - 1),
                                        skip_group_check=(e > 0),
                                    )
                                nc.scalar.activation(
                                    dst[ft][:, sl], ps[:], AF.Identity,
                                    bias=b_sb[:, ft:ft + 1],
                                )
                        for st4 in range(4):
                            st = ss * 4 + st4
                            ps = psB.tile([128, F], f32, tag="vproj",
                                          name="psv")
                            for e in range(NE):
                                nc.tensor.matmul(
                                    ps[:],
                                    xt[e][:, st4 * 128:(st4 + 1) * 128],
                                    wTv[e][:],
                                    start=(e == 0), stop=False,
                                    skip_group_check=(e > 0),
                                )
                            nc.tensor.matmul(
                                ps[:], ones_sb, bv_sb,
                                start=False, stop=True, skip_group_check=True,
                            )
                            nc.scalar.copy(v_sb[st][:], ps[:])

                # ---------- Phase 2+3: attention + out-projection -----------
                with (
                    tc.tile_pool(name=f"pscore{_rep}", bufs=1,
                                 space="PSUM") as pscore,
                    tc.tile_pool(name=f"pctx{_rep}", bufs=1,
                                 space="PSUM") as pctx,
                    tc.tile_pool(name=f"pout{_rep}", bufs=2,
                                 space="PSUM") as pout,
                    tc.tile_pool(name=f"atp{_rep}", bufs=4) as at_pool,
                    tc.tile_pool(name=f"op{_rep}", bufs=4) as o_pool,
                ):
                    for ss in (0, 2, 1, 3):
                        sl = slice(ss * 512, (ss + 1) * 512)
                        for hp in range(NF):
                            psc = [pctx.tile([64, 512], f32, tag=f"ctx{p}",
                                             name=f"psctx{p}")
                                   for p in range(2)]
                            for g in range(NST // GRP):
                                pss_p, at_p = [], []
                                for p in range(2):
                                    rows = slice(p * 64, (p + 1) * 64)
                                    pss = pscore.tile([128, 512 * GRP], f32,
                                                      tag=f"score{p}",
                                                      name=f"psscore{p}")
                                    for jj in range(GRP):
                                        j = GRP * g + jj
                                        nc.tensor.matmul(
                                            pss[:, jj * 512:(jj + 1) * 512],
                                            k_sb[hp][rows,
                                                     j * 128:(j + 1) * 128],
                                            q_sb[hp][rows, sl],
                                            start=True, stop=True,
                                        )
                                    pss_p.append(pss)
                                for p in range(2):
                                    at_t = at_pool.tile([128, 512 * GRP], f16,
                                                        tag=f"attn{p}",
                                                        name=f"attnT{p}")
                                    _emit_softsign(nc, at_t[:], pss_p[p][:])
                                    at_p.append(at_t)
                                for jj in range(GRP):
                                    j = GRP * g + jj
                                    for p in range(2):
                                        h = 2 * hp + p
                                        nc.tensor.matmul(
                                            psc[p][:],
                                            v_sb[j][:, h * 64:(h + 1) * 64],
                                            at_p[p][:,
                                                    jj * 512:(jj + 1) * 512],
                                            start=(g == 0 and jj == 0),
                                            stop=(g == NST // GRP - 1
                                                  and jj == GRP - 1),
                                            skip_group_check=not (
                                                g == 0 and jj == 0),
                                        )
                            for p in range(2):
                                rows = slice(p * 64, (p + 1) * 64)
                                nc.scalar.copy(ctx_sb[hp][rows, sl], psc[p][:])
                        for st4 in range(4):
                            st = ss * 4 + st4
                            for eh in range(2):
                                esl = slice(eh * 512, (eh + 1) * 512)
                                pso = pout.tile([128, 512], f32, tag="out",
                                                name="psout")
                                for hp in range(NF):
                                    nc.tensor.matmul(
                                        pso[:],
                                        ctx_sb[hp][:, ss * 512 + st4 * 128:
                                                   ss * 512 + (st4 + 1) * 128],
                                        woT[hp][:, esl],
                                        start=(hp == 0), stop=False,
                                        skip_group_check=(hp > 0),
                                    )
                                nc.tensor.matmul(
                                    pso[:], ones_sb, bo_sb[:, esl],
                                    start=False, stop=True,
                                    skip_group_check=True,
                                )
                                o_t = o_pool.tile([128, 512], f16, tag="ot",
                                                  name="otile")
                                nc.scalar.copy(o_t[:], pso[:])
                                nc.scalar.dma_start(
                                    ob[ss][st4 * 128:(st4 + 1) * 128, esl],
                                    o_t[:],
                                )
                        nc.gpsimd.collective_compute(
                            "ReduceScatter", ADD, PAIRS,
                            ins=[ob[ss].opt()], outs=[obr[ss].opt()])
                        nc.scalar.dma_start(
                            out_d[ss * 256:(ss + 1) * 256, :], obr[ss][:])

    mybir.codegen_inst_isa_subclasses(nc)
    _split_excess_waits(nc, 1)
    return nc


_NC_CACHE = None


def _get_nc():
    global _NC_CACHE
    if _NC_CACHE is None:
        _NC_CACHE = _build()
    return _NC_CACHE


def make_in_maps(x, Wq, bq, Wk, bk, Wv, bv, Wo, bo):
    """Per-core input dicts for cores 0..7 (core = 2*b + hg).

    Casts are fused into direct writes of each core's blob (single pass
    over every source byte - no full-array fp16 intermediates)."""
    x = np.asarray(x)
    Wq, Wk, Wv, Wo = (np.asarray(a) for a in (Wq, Wk, Wv, Wo))
    bqf = np.asarray(bq).astype(np.float32) * np.float32(0.125)
    bkf = np.asarray(bk).astype(np.float32)
    bvv = np.asarray(bv)
    boh = (np.asarray(bo).astype(np.float32) * 0.5).astype(np.float16)
    in_maps = []
    for c in range(8):
        b, hg = divmod(c, 2)
        f0 = hg * 512
        rs = slice(f0 + 128 * b, f0 + 128 * (b + 1))
        blob = np.empty((1540, E), np.float16)
        blob[0:128] = Wq[rs] * np.float32(0.125)
        blob[128:256] = Wk[rs]
        blob[256:384] = Wv[rs]
        blob[384:512] = Wo[256 * b: 256 * (b + 1),
                           f0: f0 + 512].reshape(128, E)
        blob[512:1536] = x[b, hg * SH: (hg + 1) * SH]
        bqk = np.ascontiguousarray(np.concatenate([
            bqf[f0: f0 + 512].reshape(4, 128).T,
            bkf[f0: f0 + 512].reshape(4, 128).T], axis=1))
        blob[1536:1538] = bqk.reshape(-1).view(np.float16).reshape(2, E)
        blob[1538:1540] = 0
        blob[1538, 0:128] = 1.0
        blob[1538, 128:640] = bvv[f0: f0 + 512]
        blob[1538, 640:1024] = boh[0:384]
        blob[1539, 0:640] = boh[384:1024]
        in_maps.append({"blob": blob})
    return in_maps


def kernel(x, Wq, bq, Wk, bk, Wv, bv, Wo, bo):
    nc = _get_nc()
    in_maps = make_in_maps(x, Wq, bq, Wk, bk, Wv, bv, Wo, bo)
    res = None
    for attempt in range(3):
        try:
            res = run_bass_kernel_spmd(nc, in_maps, list(range(8))).results
            break
        except Exception:
            # transient runtime/tunnel failures ("notify failed", worker
            # hang-up) recover on retry; re-raise only if persistent
            if attempt == 2:
                raise
            import time as _time
            _time.sleep(2.0)
    out = np.empty((4, S, E), np.float32)
    for b in range(4):
        for m in range(2):
            r = res[2 * b + m]["out"].astype(np.float32)      # [SH, E]
            for ss in range(NS):
                out[b, 512 * ss + 256 * m: 512 * ss + 256 * (m + 1)] = \
                    r[256 * ss: 256 * (ss + 1)]
    return (out,)


if __name__ == "__main__":
    rng = np.random.RandomState(0)
    s = 1.0 / np.sqrt(E)
    inputs = dict(
        x=rng.randn(4, S, E).astype(np.float32),
        Wq=rng.uniform(-s, s, (E, E)).astype(np.float32),
        bq=rng.uniform(-s, s, E).astype(np.float32),
        Wk=rng.uniform(-s, s, (E, E)).astype(np.float32),
        bk=rng.uniform(-s, s, E).astype(np.float32),
        Wv=rng.uniform(-s, s, (E, E)).astype(np.float32),
        bv=rng.uniform(-s, s, E).astype(np.float32),
        Wo=rng.uniform(-s, s, (E, E)).astype(np.float32),
        bo=rng.uniform(-s, s, E).astype(np.float32),
    )
    out = kernel(**inputs)[0]

    # numpy reference
    x, Wq, bq, Wk, bk = (inputs[k] for k in ("x", "Wq", "bq", "Wk", "bk"))
    Wv, bv, Wo, bo = (inputs[k] for k in ("Wv", "bv", "Wo", "bo"))
    B, H = 4, 16
    q = (x @ Wq.T + bq).reshape(B, S, H, D).transpose(0, 2, 1, 3)
    k = (x @ Wk.T + bk).reshape(B, S, H, D).transpose(0, 2, 1, 3)
    v = (x @ Wv.T + bv).reshape(B, S, H, D).transpose(0, 2, 1, 3)
    sc = np.einsum("bhqd,bhkd->bhqk", q, k) / np.sqrt(np.float32(D))
    at = sc / (1 + np.abs(sc))
    ctx = np.einsum("bhqk,bhkd->bhqd", at, v)
    ref = ctx.transpose(0, 2, 1, 3).reshape(B, S, E) @ Wo.T + bo
    err = np.abs(out - ref)
    print("out", out.shape, out.dtype, "max abs err", err.max(),
          "rel", err.max() / np.abs(ref).max())
